# revision 9
# baseline (speedup 1.0000x reference)
"""Trainium2 Bass kernel for IR-Net style binarized 3x3 conv + BN + Hardtanh.

Reference computation:
  bw = sign(standardize(weight)) * sw   (sw = per-cout power-of-2 scale)
  ba = sign(x)
  y  = clip(conv3x3(ba, bw) * bn_scale + bn_bias, -1, 1)

Both matmul operands are exactly +-1 (exactly representable in fp8e4m3),
so the conv runs as fp8 DoubleRow matmuls on the TensorEngine with zero
numerical error (fp32 PSUM accumulation of integers <= 2304).

All data prep is host-side: weights are standardized/signed/packed, and
the activations are binarized and packed into zero-padded fp8 planes on
the host, so the device does ONLY matmuls + epilogue + stores.  The
padded plane uses a 57-element row stride: one shared zero column serves
as both the right pad of row r and the left pad of row r+1, so the 3x3
conv over an 8-row block is 9 accumulated DoubleRow matmuls over shifted
windows of 456 columns (vs 464 with separate pads).

Epilogue: ScalarEngine ACT applies the folded BN scale/bias straight out
of PSUM (bf16 out), VectorEngine clips to [-1,1], and the store DMA
writes bf16 (exact for all surviving values: unclipped outputs come from
small-integer conv sums).  Garbage columns (the shared zero col) are
stored and stripped on the host.

Distribution: pure data parallel, 32 images -> 4 per NeuronCore, full
weights replicated, no collectives.
"""

import numpy as np

import concourse.bass as bass
import concourse.bacc as bacc
import concourse.mybir as mybir
import concourse.tile as tile
from concourse.bass_utils import run_bass_kernel_spmd

B, CIN, COUT, H, W = 32, 256, 256, 56, 56
NCORES = 8
BPC = B // NCORES            # images per core
S = 57                       # plane row stride: 56 px + 1 shared zero col
BASE = 58                    # plane offset of input (row0, col0); 58 lead zeros
XT = 3312                    # plane elems per cin-chunk (%16==0)
NCI = CIN // 128             # 2 cin chunks = DoubleRow k-subtiles
NCO = COUT // 128            # 2 cout chunks
RB = 8                       # output rows per tile
NBLK = H // RB               # 7
NT = RB * S                  # 456 matmul free dim (incl. 8 garbage cols)
KTAPS = 9
BN_EPS = 1e-5

CH0 = 576                    # img0 first chunk (covers all of block0's reads)
CHK = 456                    # img0 follow-up chunks: 6 x 456 elems

F32 = mybir.dt.float32
BF16 = mybir.dt.bfloat16
FP8 = mybir.dt.float8e4

_CACHE: dict = {}


def _build_nc() -> bass.Bass:
    nc = bacc.Bacc("TRN2", target_bir_lowering=False, debug=False, num_devices=NCORES)
    xin = nc.declare_dram_parameter("xin", [BPC, 128, XT * NCI], FP8, isOutput=False)
    wts = nc.declare_dram_parameter(
        "wts", [128, NCO * KTAPS * NCI * 128], FP8, isOutput=False
    )
    sb = nc.declare_dram_parameter("sb", [128, 2 * NCO], F32, isOutput=False)
    yout = nc.declare_dram_parameter(
        "yout", [BPC, NCO, 128, NBLK * NT], BF16, isOutput=True
    )

    with tile.TileContext(nc) as tc:
        with (
            tc.tile_pool(name="const", bufs=1) as cpool,
            tc.tile_pool(name="psum", bufs=8, space=bass.MemorySpace.PSUM) as ppool,
            tc.tile_pool(name="ob", bufs=6) as obpool,
            tc.tile_pool(name="oc", bufs=6) as ocpool,
        ):
            # weights: [p, (co, k, j, m)]; (co0,k0) split out so the first
            # LDWEIGHTS only waits on a 32KB DMA.
            w_sb = cpool.tile([128, NCO * KTAPS * NCI * 128], FP8, tag="w")
            sb_sb = cpool.tile([128, 2 * NCO], F32, tag="sb")
            HWCO = KTAPS * NCI * 128  # 2304 elems per cout chunk
            nc.scalar.dma_start(w_sb[:, 0:256], wts[:, 0:256])  # (co0,k0): LDW0 gate
            nc.scalar.dma_start(w_sb[:, 256:HWCO], wts[:, 256:HWCO])
            nc.scalar.dma_start(sb_sb[:], sb[:])
            w2d = nc.scalar.dma_start(w_sb[:, HWCO:], wts[:, HWCO:])
            w4 = w_sb.rearrange("p (co k j m) -> p co k j m", co=NCO, k=KTAPS, j=NCI)

            # Host-packed padded fp8 planes, cin chunks byte-interleaved as
            # the innermost dim (DoubleRow k-subtiles); DMA straight in.
            xp = {}
            for img in range(BPC):
                t = cpool.tile([128, XT, NCI], FP8, tag=f"xp{img}")
                xp[img] = t

            # img0 chunked across two queues (sync + vector) so chunk
            # completions outpace the block cadence: chunk m ready => block m
            # ready.
            bounds = [0, CH0] + [CH0 + CHK * (k + 1) for k in range(6)]
            c0d = []
            for i, (a, b) in enumerate(zip(bounds[:-1], bounds[1:])):
                d = nc.sync.dma_start(xp[0][:, a:b, :], xin[0, :, a * NCI : b * NCI])
                c0d.append(d)
            # co1 weights aren't needed until img0/co1 (~22us in): keep the
            # big weight DMA off the wire during the critical first chunks.
            tile.add_dep_helper(
                w2d.ins,
                c0d[1].ins,
                sync=True,
                reason="delay co1 weights behind img0 critical chunks",
            )
            # imgs 1..3: whole-plane loads on gpsimd, staggered behind img0's
            # chunks so those get full HBM bandwidth.
            for img in range(1, BPC):
                d = nc.gpsimd.dma_start(xp[img][:, :, :], xin[img, :, :])
                tile.add_dep_helper(
                    d.ins,
                    c0d[min(img + 3, 6)].ins,
                    sync=True,
                    reason="stagger bulk input loads behind img0 critical path",
                )

            for img in range(BPC):
                for co in range(NCO):
                    s_ap = sb_sb[:, co : co + 1]
                    b_ap = sb_sb[:, NCO + co : NCO + co + 1]
                    blocks = [(bk * RB, RB) for bk in range(NBLK)]
                    if img == BPC - 1 and co == NCO - 1:
                        # split the final tile: shortest possible tail after
                        # the last matmul
                        blocks = blocks[:-1] + [(48, 4), (52, 4)]
                    for r0, rb in blocks:
                        nt = rb * S
                        ps = ppool.tile([128, nt], F32, tag="ps")
                        for k in range(KTAPS):
                            ky, kx = divmod(k, 3)
                            s0 = BASE + (r0 + ky - 1) * S + (kx - 1)
                            rhs = xp[img][:, s0 : s0 + nt, :].rearrange(
                                "p x j -> p j x"
                            )
                            nc.tensor.matmul(
                                ps[:],
                                w4[:, co, k],
                                rhs,
                                start=(k == 0),
                                stop=(k == KTAPS - 1),
                                perf_mode=mybir.MatmulPerfMode.DoubleRow,
                            )
                        # BN affine on ScalarE straight out of PSUM (bf16 out)
                        ob = obpool.tile([128, nt], BF16, tag="ob")
                        nc.scalar.activation(
                            ob[:],
                            ps[:],
                            mybir.ActivationFunctionType.Identity,
                            bias=b_ap,
                            scale=s_ap,
                        )
                        # hardtanh on VectorE (bf16 in/out: 2x DVE rate)
                        oc = ocpool.tile([128, nt], BF16, tag="oc")
                        nc.vector.tensor_scalar(
                            oc[:],
                            ob[:],
                            -1.0,
                            1.0,
                            op0=mybir.AluOpType.max,
                            op1=mybir.AluOpType.min,
                        )
                        nc.sync.dma_start(
                            yout[img, co, :, r0 * S : r0 * S + nt], oc[:]
                        )
    nc.finalize()
    return nc


def get_nc() -> bass.Bass:
    if "nc" not in _CACHE:
        _CACHE["nc"] = _build_nc()
    return _CACHE["nc"]


def _host_prep(weight, gamma, beta, running_mean, running_var):
    """Binarize standardized weights, fold sw + BN into scale/bias."""
    wf = weight.reshape(COUT, -1).astype(np.float64)
    n = wf.shape[1]
    mean = wf.mean(axis=1, keepdims=True)
    d = wf - mean
    sgn = np.where(d >= 0, 1.0, -1.0)
    std = np.sqrt((d * d).sum(axis=1, keepdims=True) / (n - 1))
    bw = d / std
    sw = np.exp2(np.round(np.log2(np.abs(bw).mean(axis=1))))  # [COUT]
    inv = gamma.astype(np.float64) / np.sqrt(running_var.astype(np.float64) + BN_EPS)
    scale = (sw * inv).astype(np.float32)
    bias = (beta.astype(np.float64) - running_mean.astype(np.float64) * inv).astype(
        np.float32
    )

    fp8np = mybir.dt.np(FP8)
    # wts[p, (co, k, j, m)] = sgn[co*128+m, (j*128+p)*9 + k]
    w6 = sgn.reshape(NCO, 128, NCI, 128, KTAPS)  # [co, m, j, p, k]
    wts = (
        np.ascontiguousarray(np.transpose(w6, (3, 0, 4, 2, 1)))  # p co k j m
        .reshape(128, NCO * KTAPS * NCI * 128)
        .astype(fp8np)
    )
    # sb[m, co] = scale chunk, sb[m, NCO+co] = bias chunk
    sbarr = np.concatenate(
        [scale.reshape(NCO, 128).T, bias.reshape(NCO, 128).T], axis=1
    ).astype(np.float32)
    return np.ascontiguousarray(wts), np.ascontiguousarray(sbarr)


def _pack_x(x):
    """sign(x) -> zero-padded fp8 planes [B, 128, XT, NCI] (j interleaved)."""
    fp8np = mybir.dt.np(FP8)
    sgn = np.sign(x).astype(fp8np)  # [B, 256, 56, 56]; sign(0)=0 as reference
    sgnr = sgn.reshape(B, NCI, 128, H, W)
    buf = np.zeros((B, 128, XT, NCI), dtype=fp8np)
    region = buf[:, :, BASE : BASE + H * S, :].reshape(B, 128, H, S, NCI)
    region[:, :, :, :W, :] = sgnr.transpose(0, 2, 3, 4, 1)
    return buf


def run(x, weight, gamma, beta, running_mean, running_var, trace=False, **tkw):
    x = np.asarray(x, dtype=np.float32)
    wts, sbarr = _host_prep(
        np.asarray(weight, dtype=np.float32),
        np.asarray(gamma, dtype=np.float32),
        np.asarray(beta, dtype=np.float32),
        np.asarray(running_mean, dtype=np.float32),
        np.asarray(running_var, dtype=np.float32),
    )
    xb = _pack_x(x).reshape(B, 128, XT * NCI)
    in_maps = [
        {
            "xin": xb[c * BPC : (c + 1) * BPC],
            "wts": wts,
            "sb": sbarr,
        }
        for c in range(NCORES)
    ]
    nc = get_nc()
    res = run_bass_kernel_spmd(nc, in_maps, list(range(NCORES)), trace=trace, **tkw)
    y = np.concatenate([r["yout"] for r in res.results], axis=0)  # [B,NCO,128,3192]
    y = y.reshape(B, COUT, H, S)[..., :W].astype(np.float32)
    return np.ascontiguousarray(y), res


def kernel(x, weight, gamma, beta, running_mean, running_var):
    y, _ = run(x, weight, gamma, beta, running_mean, running_var)
    return y


# revision 11
# speedup vs baseline: 1.1987x; 1.1987x over previous
"""Trainium2 Bass kernel for IR-Net style binarized 3x3 conv + BN + Hardtanh.

Reference computation:
  bw = sign(standardize(weight)) * sw   (sw = per-cout power-of-2 scale)
  ba = sign(x)
  y  = clip(conv3x3(ba, bw) * bn_scale + bn_bias, -1, 1)

Both matmul operands are exactly +-1 (exactly representable in fp8e4m3),
so the conv runs as fp8 DoubleRow matmuls on the TensorEngine with zero
numerical error (fp32 PSUM accumulation of integers <= 2304).

All data prep is host-side: weights are standardized/signed/packed, and
the activations are binarized and packed into zero-padded fp8 planes on
the host, so the device does ONLY matmuls + epilogue + stores.  The
padded plane uses a 57-element row stride: one shared zero column serves
as both the right pad of row r and the left pad of row r+1, so the 3x3
conv over an 8-row block is 9 accumulated DoubleRow matmuls over shifted
windows of 456 columns (vs 464 with separate pads).

Epilogue: ScalarEngine ACT applies the folded BN scale/bias straight out
of PSUM (bf16 out), VectorEngine clips to [-1,1], and the store DMA
writes bf16 (exact for all surviving values: unclipped outputs come from
small-integer conv sums).  Garbage columns (the shared zero col) are
stored and stripped on the host.

Distribution: pure data parallel, 32 images -> 4 per NeuronCore, full
weights replicated, no collectives.
"""

import numpy as np

import concourse.bass as bass
import concourse.bacc as bacc
import concourse.mybir as mybir
import concourse.tile as tile
from concourse.bass_utils import run_bass_kernel_spmd

B, CIN, COUT, H, W = 32, 256, 256, 56, 56
NCORES = 8
BPC = B // NCORES            # images per core
S = 57                       # plane row stride: 56 px + 1 shared zero col
BASE = 58                    # plane offset of input (row0, col0); 58 lead zeros
XT = 3312                    # plane elems per cin-chunk (%16==0)
NCI = CIN // 128             # 2 cin chunks = DoubleRow k-subtiles
NCO = COUT // 128            # 2 cout chunks
RB = 8                       # output rows per tile
NBLK = H // RB               # 7
NT = RB * S                  # 456 matmul free dim (incl. 8 garbage cols)
KTAPS = 9
BN_EPS = 1e-5

CH0 = 576                    # img0 first chunk (covers all of block0's reads)
CHK = 456                    # img0 follow-up chunks: 6 x 456 elems

F32 = mybir.dt.float32
BF16 = mybir.dt.bfloat16
FP8 = mybir.dt.float8e4

_CACHE: dict = {}


def _build_nc() -> bass.Bass:
    nc = bacc.Bacc("TRN2", target_bir_lowering=False, debug=False, num_devices=NCORES)
    xin = nc.declare_dram_parameter("xin", [BPC, 128, XT * NCI], FP8, isOutput=False)
    wts = nc.declare_dram_parameter(
        "wts", [128, NCO * KTAPS * NCI * 128], FP8, isOutput=False
    )
    sb = nc.declare_dram_parameter("sb", [128, 2 * NCO], F32, isOutput=False)
    yout = nc.declare_dram_parameter(
        "yout", [BPC, NCO, 128, NBLK * NT], BF16, isOutput=True
    )

    with tile.TileContext(nc) as tc:
        with (
            tc.tile_pool(name="const", bufs=1) as cpool,
            tc.tile_pool(name="psum", bufs=7, space=bass.MemorySpace.PSUM) as ppool,
            tc.tile_pool(name="dpsum", bufs=1, space=bass.MemorySpace.PSUM) as dpool,
            tc.tile_pool(name="ob", bufs=6) as obpool,
            tc.tile_pool(name="oc", bufs=6) as ocpool,
        ):
            # weights: [p, (co, k, j, m)]; (co0,k0) split out so the first
            # LDWEIGHTS only waits on a 32KB DMA.
            w_sb = cpool.tile([128, NCO * KTAPS * NCI * 128], FP8, tag="w")
            sb_sb = cpool.tile([128, 2 * NCO], F32, tag="sb")
            HWCO = KTAPS * NCI * 128  # 2304 elems per cout chunk
            nc.scalar.dma_start(w_sb[:, 0:HWCO], wts[:, 0:HWCO])
            nc.scalar.dma_start(sb_sb[:], sb[:])
            w2d = nc.scalar.dma_start(w_sb[:, HWCO:], wts[:, HWCO:])

            # Warmup: the PE clock ramps over ~4.5us (~1 mm at 3.3x + 10 at
            # 2x before full speed).  The PE is idle during the input-DMA
            # prologue anyway, so burn that window with dummy matmuls on an
            # unread PSUM bank; block0 then runs at full clock.
            dum = cpool.tile([128, 1024], FP8, tag="dum")
            nc.vector.memset(dum[:], 0.0)
            dps = dpool.tile([128, 456], F32, tag="dps")
            NWARM = 12
            for i in range(NWARM):
                nc.tensor.matmul(
                    dps[:],
                    dum[:, 0:256].rearrange("p (j m) -> p j m", j=2),
                    dum[:, 0:912].rearrange("p (j x) -> p j x", j=2),
                    start=(i == 0),
                    stop=(i == NWARM - 1),
                    perf_mode=mybir.MatmulPerfMode.DoubleRow,
                )
            w4 = w_sb.rearrange("p (co k j m) -> p co k j m", co=NCO, k=KTAPS, j=NCI)

            # Host-packed padded fp8 planes, cin chunks byte-interleaved as
            # the innermost dim (DoubleRow k-subtiles); DMA straight in.
            xp = {}
            for img in range(BPC):
                t = cpool.tile([128, XT, NCI], FP8, tag=f"xp{img}")
                xp[img] = t

            # img0 chunked across two queues (sync + vector) so chunk
            # completions outpace the block cadence: chunk m ready => block m
            # ready.
            bounds = [0, CH0] + [CH0 + CHK * (k + 1) for k in range(6)]
            c0d = []
            for i, (a, b) in enumerate(zip(bounds[:-1], bounds[1:])):
                d = nc.sync.dma_start(xp[0][:, a:b, :], xin[0, :, a * NCI : b * NCI])
                c0d.append(d)
            # co1 weights aren't needed until img0/co1 (~22us in): keep the
            # big weight DMA off the wire during the critical first chunks.
            tile.add_dep_helper(
                w2d.ins,
                c0d[1].ins,
                sync=True,
                reason="delay co1 weights behind img0 critical chunks",
            )
            # imgs 1..3: whole-plane loads on gpsimd, staggered behind img0's
            # chunks so those get full HBM bandwidth.
            for img in range(1, BPC):
                d = nc.gpsimd.dma_start(xp[img][:, :, :], xin[img, :, :])
                tile.add_dep_helper(
                    d.ins,
                    c0d[min(img + 3, 6)].ins,
                    sync=True,
                    reason="stagger bulk input loads behind img0 critical path",
                )

            for img in range(BPC):
                for co in range(NCO):
                    s_ap = sb_sb[:, co : co + 1]
                    b_ap = sb_sb[:, NCO + co : NCO + co + 1]
                    blocks = [(bk * RB, RB) for bk in range(NBLK)]
                    if img == BPC - 1 and co == NCO - 1:
                        # split the final tile: shortest possible tail after
                        # the last matmul
                        blocks = blocks[:-1] + [(48, 4), (52, 4)]
                    for r0, rb in blocks:
                        nt = rb * S
                        ps = ppool.tile([128, nt], F32, tag="ps")
                        for k in range(KTAPS):
                            ky, kx = divmod(k, 3)
                            s0 = BASE + (r0 + ky - 1) * S + (kx - 1)
                            rhs = xp[img][:, s0 : s0 + nt, :].rearrange(
                                "p x j -> p j x"
                            )
                            nc.tensor.matmul(
                                ps[:],
                                w4[:, co, k],
                                rhs,
                                start=(k == 0),
                                stop=(k == KTAPS - 1),
                                perf_mode=mybir.MatmulPerfMode.DoubleRow,
                            )
                        # BN affine on ScalarE straight out of PSUM (bf16 out)
                        ob = obpool.tile([128, nt], BF16, tag="ob")
                        nc.scalar.activation(
                            ob[:],
                            ps[:],
                            mybir.ActivationFunctionType.Identity,
                            bias=b_ap,
                            scale=s_ap,
                        )
                        # hardtanh on VectorE (bf16 in/out: 2x DVE rate)
                        oc = ocpool.tile([128, nt], BF16, tag="oc")
                        nc.vector.tensor_scalar(
                            oc[:],
                            ob[:],
                            -1.0,
                            1.0,
                            op0=mybir.AluOpType.max,
                            op1=mybir.AluOpType.min,
                        )
                        nc.sync.dma_start(
                            yout[img, co, :, r0 * S : r0 * S + nt], oc[:]
                        )
    nc.finalize()
    return nc


def get_nc() -> bass.Bass:
    if "nc" not in _CACHE:
        _CACHE["nc"] = _build_nc()
    return _CACHE["nc"]


def _host_prep(weight, gamma, beta, running_mean, running_var):
    """Binarize standardized weights, fold sw + BN into scale/bias."""
    wf = weight.reshape(COUT, -1).astype(np.float64)
    n = wf.shape[1]
    mean = wf.mean(axis=1, keepdims=True)
    d = wf - mean
    sgn = np.where(d >= 0, 1.0, -1.0)
    std = np.sqrt((d * d).sum(axis=1, keepdims=True) / (n - 1))
    bw = d / std
    sw = np.exp2(np.round(np.log2(np.abs(bw).mean(axis=1))))  # [COUT]
    inv = gamma.astype(np.float64) / np.sqrt(running_var.astype(np.float64) + BN_EPS)
    scale = (sw * inv).astype(np.float32)
    bias = (beta.astype(np.float64) - running_mean.astype(np.float64) * inv).astype(
        np.float32
    )

    fp8np = mybir.dt.np(FP8)
    # wts[p, (co, k, j, m)] = sgn[co*128+m, (j*128+p)*9 + k]
    w6 = sgn.reshape(NCO, 128, NCI, 128, KTAPS)  # [co, m, j, p, k]
    wts = (
        np.ascontiguousarray(np.transpose(w6, (3, 0, 4, 2, 1)))  # p co k j m
        .reshape(128, NCO * KTAPS * NCI * 128)
        .astype(fp8np)
    )
    # sb[m, co] = scale chunk, sb[m, NCO+co] = bias chunk
    sbarr = np.concatenate(
        [scale.reshape(NCO, 128).T, bias.reshape(NCO, 128).T], axis=1
    ).astype(np.float32)
    return np.ascontiguousarray(wts), np.ascontiguousarray(sbarr)


def _pack_x(x):
    """sign(x) -> zero-padded fp8 planes [B, 128, XT, NCI] (j interleaved)."""
    fp8np = mybir.dt.np(FP8)
    sgn = np.sign(x).astype(fp8np)  # [B, 256, 56, 56]; sign(0)=0 as reference
    sgnr = sgn.reshape(B, NCI, 128, H, W)
    buf = np.zeros((B, 128, XT, NCI), dtype=fp8np)
    region = buf[:, :, BASE : BASE + H * S, :].reshape(B, 128, H, S, NCI)
    region[:, :, :, :W, :] = sgnr.transpose(0, 2, 3, 4, 1)
    return buf


def run(x, weight, gamma, beta, running_mean, running_var, trace=False, **tkw):
    x = np.asarray(x, dtype=np.float32)
    wts, sbarr = _host_prep(
        np.asarray(weight, dtype=np.float32),
        np.asarray(gamma, dtype=np.float32),
        np.asarray(beta, dtype=np.float32),
        np.asarray(running_mean, dtype=np.float32),
        np.asarray(running_var, dtype=np.float32),
    )
    xb = _pack_x(x).reshape(B, 128, XT * NCI)
    in_maps = [
        {
            "xin": xb[c * BPC : (c + 1) * BPC],
            "wts": wts,
            "sb": sbarr,
        }
        for c in range(NCORES)
    ]
    nc = get_nc()
    res = run_bass_kernel_spmd(nc, in_maps, list(range(NCORES)), trace=trace, **tkw)
    y = np.concatenate([r["yout"] for r in res.results], axis=0)  # [B,NCO,128,3192]
    y = y.reshape(B, COUT, H, S)[..., :W].astype(np.float32)
    return np.ascontiguousarray(y), res


def kernel(x, weight, gamma, beta, running_mean, running_var):
    y, _ = run(x, weight, gamma, beta, running_mean, running_var)
    return y
